# revision 11
# baseline (speedup 1.0000x reference)
"""Causal self-attention (B=4, T=2048, C=1024, H=16) on 8 TRN2 NeuronCores.

Sharding: core = (batch b, head-group g) with b in 0..3, g in 0..1.
Each core handles one batch element and 8 of the 16 heads (tensor-parallel
split of the QKV / proj weights).  The c_proj contraction is split over the
two head groups, so each core produces a partial [T, C] output; the host
sums the two partials per batch and adds b_proj (the "all-reduce" of the
TP sharding, done on the host during unsharding).

Device layout (per core) -- everything SBUF-resident, bf16 matmul inputs:
  xT    [C, T]        x[b]^T, host-transposed + bf16-cast
  wqkv  [C, 3*CL]     W_attn column slice for this head group (q scaled by
                      1/sqrt(D) on host), bf16
  qkvT = wqkv.T @ xT computed as [ch, t] tiles (q^T, k^T); v computed in
  natural [t, d] orientation as xT.T @ wv.
  Attention per head h: S^T[j, i] = k^T.T q^T  (contraction d=64, two heads
  packed on PE row-groups 0-63 / 64-127), structural causal masking (only
  lower-triangular j-tiles computed; diagonal tiles get an additive -1e9
  triangle constant), exp on ScalarE (no max subtraction -- scores are O(6)),
  P^T @ V' on PE with V' = [V | ones] so row 64 of the accumulator is the
  softmax denominator.  Normalization happens on the PSUM->SBUF copy.
  c_proj: out[t, :] += y^T.T @ Wp with K=128 channel tiles.
"""

import math

import ml_dtypes
import numpy as np

import concourse.bass as bass
import concourse.tile as tile
from concourse import bacc, mybir
from concourse.bass_utils import run_bass_kernel_spmd

# problem shape (hardcoded per the task contract)
B, T, C, H = 4, 2048, 1024, 16
D = C // H            # 64 head dim
NCORES = 8
HL = H // 2           # heads per core
CL = HL * D           # 512 local channels per core
NEG = -1.0e9

P = 128               # SBUF partitions
TI = 512              # query chunk (matmul moving dim)
TJ = 128              # key tile
CT = C // P           # 8 contraction tiles for the projections
NTT = T // P          # 16 t-tiles of 128
NIT = T // TI         # 4 query chunks
JQ = CL // P          # 4 channel tiles for q (and for k, and for y)
KC = CL // P          # 4 channel tiles in c_proj contraction
NOC = C // TI         # 2 output-column tiles in c_proj

FP32 = mybir.dt.float32
BF16 = mybir.dt.bfloat16
AF = mybir.ActivationFunctionType
ADD = mybir.AluOpType.add
MULT = mybir.AluOpType.mult


def _emit(tc, io):
    nc = tc.nc
    xT, wqkv, bqk, bv, wp, mtri, idn, out = (
        io["xT"], io["wqkv"], io["bqk"], io["bv"], io["wp"], io["mtri"],
        io["idn"], io["out"]
    )

    with (
        tc.tile_pool(name="const", bufs=1) as cpool,
        tc.tile_pool(name="work", bufs=4) as wpool,
        tc.tile_pool(name="outp", bufs=6) as opool,
        tc.tile_pool(name="mm", bufs=2, space="PSUM") as mm_ps,
        tc.tile_pool(name="ps", bufs=2, space="PSUM") as s_ps,
        tc.tile_pool(name="po", bufs=2, space="PSUM") as o_ps,
    ):
        # persistent SBUF tensors
        xT_sb = cpool.tile([P, CT, T], BF16)
        wqkv_sb = cpool.tile([P, CT, 3 * CL], BF16)
        qT_sb = cpool.tile([P, JQ, T], BF16)
        kT_sb = cpool.tile([P, JQ, T], BF16)
        v_sb = cpool.tile([P, NTT, HL, D + 1], BF16)
        yT_sb = cpool.tile([P, JQ, T], BF16)
        wp_sb = cpool.tile([P, KC, C], BF16)
        mtri_sb = cpool.tile([P, P], BF16)
        idn_sb = cpool.tile([P, P], BF16)
        bqk_sb = cpool.tile([P, 2 * JQ], FP32)
        bv_sb = cpool.tile([1, CL], FP32)
        wpo_sb = cpool.tile([D, 1, C], BF16)
        bvb_sb = cpool.tile([P, CL], FP32)

        xT_d = xT.rearrange("(o p) t -> p o t", p=P)
        wqkv_d = wqkv.rearrange("(o p) j -> p o j", p=P)
        # DMA queue order = first-compute order, with the very first inputs
        # split per contraction chunk o so the first QKV matmul (which only
        # needs chunk o=0) starts as soon as ~160KB has landed rather than
        # after the full 1.3MB prologue load.
        for o in range(CT):
            # partition-split so each 128KB x^T slice rides two DMA queues
            nc.sync.dma_start(xT_sb[0:D, o, 0:TI], xT_d[0:D, o, 0:TI])
            nc.sync.dma_start(xT_sb[D:P, o, 0:TI], xT_d[D:P, o, 0:TI])
            nc.sync.dma_start(wqkv_sb[:, o, 0:P], wqkv_d[:, o, 0:P])
        for o in range(CT):
            nc.sync.dma_start(
                wqkv_sb[:, o, CL : CL + P], wqkv_d[:, o, CL : CL + P]
            )
        nc.sync.dma_start(bqk_sb[:], bqk[:])
        for o in range(CT):
            nc.sync.dma_start(wqkv_sb[:, o, 2 * CL :], wqkv_d[:, o, 2 * CL :])
        nc.sync.dma_start(mtri_sb[:], mtri[:])
        nc.sync.dma_start(idn_sb[:], idn[:])
        nc.sync.dma_start(bv_sb[:], bv[:])
        for tch in range(1, NIT):
            ts = slice(tch * TI, (tch + 1) * TI)
            for o in range(CT):
                nc.sync.dma_start(xT_sb[:, o, ts], xT_d[:, o, ts])
        for o in range(CT):
            nc.sync.dma_start(wqkv_sb[:, o, P:CL], wqkv_d[:, o, P:CL])
            nc.sync.dma_start(
                wqkv_sb[:, o, CL + P : 2 * CL], wqkv_d[:, o, CL + P : 2 * CL]
            )
        wp_d = wp.rearrange("(o p) j -> p o j", p=P)
        for o in range(KC):
            nc.sync.dma_start(wp_sb[:, o, :], wp_d[:, o, :])
        # odd-head W_proj rows re-staged on partitions 0-63: the final
        # chunk's c_proj reads the odd half of y^T straight out of `tmp`
        # (partitions 0-63), skipping the cross-partition y^T DMA
        nc.sync.dma_start(
            wpo_sb[:],
            wp.rearrange("(o p) j -> p o j", p=P)[D:P, JQ - 1 :, :],
        )
        nc.gpsimd.partition_broadcast(bvb_sb[:], bv_sb[:])

        # ones column of V' (softmax denominator accumulator)
        nc.vector.memset(v_sb[:, :, :, D : D + 1], 1.0)

        wv = wqkv_sb[:, :, 2 * CL : 3 * CL]

        def emit_v_tile(tt):
            """V in natural [t, d] orientation: V = xT.T @ wv, one t-tile."""
            pv = mm_ps.tile([P, CL], FP32, tag="mm")
            for o in range(CT):
                nc.tensor.matmul(
                    pv[:],
                    xT_sb[:, o, tt * P : (tt + 1) * P],
                    wv[:, o, :],
                    start=(o == 0),
                    stop=(o == CT - 1),
                )
            # copy + v-bias (broadcast along partitions beforehand)
            nc.vector.tensor_tensor(
                v_sb[:, tt, :, 0:D],
                pv.rearrange("p (h d) -> p h d", h=HL),
                bvb_sb.rearrange("p (h d) -> p h d", h=HL),
                ADD,
            )

        def emit_qkv_group(pr, g):
            """One [128-ch, 512-t] q^T or k^T tile for pair pr."""
            which, tc_ = divmod(g, NIT)
            jt = which * JQ + pr
            dst = qT_sb if which == 0 else kT_sb
            pq = mm_ps.tile([P, TI], FP32, tag="mm")
            for o in range(CT):
                nc.tensor.matmul(
                    pq[:],
                    wqkv_sb[:, o, jt * P : (jt + 1) * P],
                    xT_sb[:, o, tc_ * TI : (tc_ + 1) * TI],
                    start=(o == 0),
                    stop=(o == CT - 1),
                )
            nc.vector.tensor_scalar_add(
                dst[:, pr, tc_ * TI : (tc_ + 1) * TI], pq[:], bqk_sb[:, jt : jt + 1]
            )

        def emit_cproj_tile(pr, idx, tail=False):
            """Pair pr's partial c_proj contribution for one [128-t, 512-c]
            output tile.  In the kernel tail the PSUM->SBUF copy alternates
            between ScalarE (idle once the exp stream ends) and DVE, and the
            DMA is split across two queues, so the final drain is not paced
            by one serial copy engine."""
            tt, oc = divmod(idx, NOC)
            pc = mm_ps.tile([P, TI], FP32, tag="mm")
            nc.tensor.matmul(
                pc[:],
                yT_sb[:, pr, tt * P : (tt + 1) * P],
                wp_sb[:, pr, oc * TI : (oc + 1) * TI],
                start=True,
                stop=True,
            )
            ob = opool.tile([P, TI], BF16, tag="ob")
            if tail and idx % 2 == 0:
                nc.scalar.activation(ob[:], pc[:], AF.Copy)
            else:
                nc.vector.tensor_copy(ob[:], pc[:])
            nsplit = 2 if tail else 1
            w = TI // nsplit
            for s in range(nsplit):
                nc.sync.dma_start(
                    out[
                        pr,
                        tt * P : (tt + 1) * P,
                        oc * TI + s * w : oc * TI + (s + 1) * w,
                    ],
                    ob[:, s * w : (s + 1) * w],
                )

        # c_proj tiles become available as pairs finish; they carry no
        # downstream dependencies, so they queue up and drain between
        # attention jt-steps to keep the PE busy (and HAM-warm) while
        # ScalarE works through the exp backlog.
        cproj_queue = []

        # keep >=10 tiles queued so the backlog can cover the final
        # normalize window (and keep HAM warm into the kernel tail)
        QFLOOR = 10

        def drain_cproj(n, floor=QFLOOR):
            for _ in range(min(n, len(cproj_queue) - floor)):
                pr_, idx = cproj_queue.pop(0)
                emit_cproj_tile(pr_, idx)

        # pair 0 prologue: just enough for attention (0, it=0); the rest of
        # pair 0's q/k/V tiles interleave at it-chunk boundaries
        emit_qkv_group(0, 0)        # q cols [0:512]
        emit_qkv_group(0, NIT)      # k cols [0:512]
        for tt in range(4):
            emit_v_tile(tt)

        for pr in range(JQ):  # 4 head pairs; pair pr = local heads 2pr, 2pr+1
            # ---- attention for the head pair (pair pr+1's q^T/k^T matmuls
            # are interleaved per it-chunk to fill PE gaps while ScalarE
            # works through the exp backlog) ----
            for step, it in enumerate(range(NIT)):
                po_e = o_ps.tile([P, TI], FP32, tag="po")
                po_o = o_ps.tile([P, TI], FP32, tag="po")
                njt = (it + 1) * (TI // TJ)

                def emit_pv(jt, p2, lo, njt=njt, po_e=po_e, po_o=po_o, pr=pr):
                    first, last = (jt == 0), (jt == njt - 1)
                    nc.tensor.matmul(
                        po_e[0 : D + 1, lo:TI],
                        v_sb[:, jt, 2 * pr, :],
                        p2[:, lo:TI],
                        start=first,
                        stop=last,
                    )
                    nc.tensor.matmul(
                        po_o[0 : D + 1, lo:TI],
                        v_sb[:, jt, 2 * pr + 1, :],
                        p2[:, TI + lo : 2 * TI],
                        start=first,
                        stop=last,
                    )

                prev = None
                for jt in range(njt):
                    delta = jt * TJ - it * TI
                    lo = max(delta, 0)
                    # merged even/odd score tile: even head in cols 0:TI
                    # (PSUM bank 0), odd head in cols TI:2*TI (bank 1)
                    ps2 = s_ps.tile([P, 2 * TI], FP32, tag="ps")
                    diag = delta >= 0
                    # S^T = k^T.T @ q^T, contraction d=64; the two heads of
                    # the pair sit on PE row groups 0-63 / 64-127 and run
                    # concurrently.
                    nc.tensor.matmul(
                        ps2[:, lo:TI],
                        kT_sb[0:D, pr, jt * TJ : (jt + 1) * TJ],
                        qT_sb[0:D, pr, it * TI + lo : (it + 1) * TI],
                        start=True,
                        stop=not diag,
                    )
                    nc.tensor.matmul(
                        ps2[:, TI + lo : 2 * TI],
                        kT_sb[D:P, pr, jt * TJ : (jt + 1) * TJ],
                        qT_sb[D:P, pr, it * TI + lo : (it + 1) * TI],
                        start=True,
                        stop=not diag,
                        tile_position=(D, 0),
                    )
                    ps2v = ps2.rearrange("p (b c) -> p b c", b=2)
                    if diag:
                        # diagonal tile: add the -1e9 strict-upper-triangle
                        # constant on the PE (identity-weight matmul
                        # accumulating mtri into PSUM) so the score->exp
                        # chain never hops through DVE.
                        nc.tensor.matmul(
                            ps2[:, delta : delta + TJ],
                            idn_sb[:],
                            mtri_sb[:],
                            start=False,
                            stop=True,
                        )
                        nc.tensor.matmul(
                            ps2[:, TI + delta : TI + delta + TJ],
                            idn_sb[:],
                            mtri_sb[:],
                            start=False,
                            stop=True,
                        )
                    p2 = wpool.tile([P, 2 * TI], BF16, tag="p")
                    p2v = p2.rearrange("p (b c) -> p b c", b=2)
                    # columns [0:lo) are fully masked and the PV matmuls
                    # only read [lo:], so exp is restricted and no memset
                    # is needed
                    if lo > 0:
                        nc.scalar.activation(
                            p2v[:, :, lo:TI], ps2v[:, :, lo:TI], AF.Exp
                        )
                    else:
                        nc.scalar.activation(p2[:], ps2[:], AF.Exp)
                    # software pipeline: scores/exp run one j-tile ahead of
                    # the PV consumer, with ready c_proj filler pumped in
                    # front of each (possibly exp-stalled) PV pair
                    if prev is not None:
                        if njt >= 8:
                            drain_cproj(1)
                        emit_pv(*prev)
                    prev = (jt, p2, lo)
                if njt >= 8:
                    drain_cproj(1)
                emit_pv(*prev)
                if pr == JQ - 1 and it == NIT - 1:
                    # ready backlog fills the PE while the final normalize
                    # chain (copies/recip/broadcast/mults) runs
                    drain_cproj(len(cproj_queue), floor=0)
                # Drain the PV accumulators to SBUF right away (frees the
                # PSUM banks for the next it-chunk), then normalize from the
                # SBUF copy: even head in osb cols 0:TI, odd head in cols
                # TI:2*TI so one reciprocal + one broadcast covers both.
                # Row D is the softmax denominator; its reciprocal runs on
                # DVE (reciprocal_approx_fast, ~51 ULP) so ScalarE stays a
                # pure exp stream.  partition_broadcast's gpsimd ucode reads
                # the source with Q7 core 0, so the reciprocal row is DMA'd
                # to partition 0 first.
                islice = slice(it * TI, (it + 1) * TI)
                is_final = pr == JQ - 1 and it == NIT - 1
                osb = wpool.tile([P, 2 * TI], FP32, tag="osb", bufs=2)
                rec = wpool.tile([1, 2 * TI], FP32, tag="rec", bufs=2)
                rcs = wpool.tile([1, 2 * TI], FP32, tag="rcs", bufs=2)
                rb = wpool.tile([P, 2 * TI], FP32, tag="rb", bufs=2)
                tmp = wpool.tile([D, TI], BF16, tag="tmp")
                # raw denominator row to partition 0 first:
                # reciprocal_approx_fast's custom-DVE lowering only works
                # at partition 0 (measured: garbage at partition 64), and
                # partition_broadcast's gpsimd ucode reads with Q7 core 0.
                # The final chunk runs the pipeline in two column halves and
                # emits each half's c_proj tiles immediately (shortens the
                # kernel tail); other chunks do it in one pass.
                nhalf = 2 if is_final else 1
                hw_ = TI // nhalf
                def emit_cproj_split(tt, oc, _cnt=[0], pr=pr, it=it, tmp=None):
                    """Final-chunk c_proj as two K=64 matmuls: the odd half
                    of y^T is read straight out of `tmp` (partitions 0-63)
                    against wpo_sb, skipping the cross-partition y^T DMA."""
                    pc = mm_ps.tile([P, TI], FP32, tag="mm")
                    nc.tensor.matmul(
                        pc[:],
                        yT_sb[0:D, pr, tt * P : (tt + 1) * P],
                        wp_sb[0:D, pr, oc * TI : (oc + 1) * TI],
                        start=True,
                        stop=False,
                    )
                    loc = tt * P - it * TI
                    nc.tensor.matmul(
                        pc[:],
                        tmp[0:D, loc : loc + P],
                        wpo_sb[:, 0, oc * TI : (oc + 1) * TI],
                        start=False,
                        stop=True,
                    )
                    ob = opool.tile([P, TI], BF16, tag="ob")
                    if _cnt[0] % 2 == 0:
                        nc.scalar.activation(ob[:], pc[:], AF.Copy)
                    else:
                        nc.vector.tensor_copy(ob[:], pc[:])
                    _cnt[0] += 1
                    w2 = TI // 2
                    for s in range(2):
                        nc.sync.dma_start(
                            out[
                                pr,
                                tt * P : (tt + 1) * P,
                                oc * TI + s * w2 : oc * TI + (s + 1) * w2,
                            ],
                            ob[:, s * w2 : (s + 1) * w2],
                        )

                for h in range(nhalf):
                    base = h * 2 * hw_
                    he = slice(base, base + hw_)          # even-head cols
                    ho = slice(base + hw_, base + 2 * hw_)  # odd-head cols
                    hfull = slice(base, base + 2 * hw_)
                    hsl = slice(it * TI + h * hw_, it * TI + (h + 1) * hw_)
                    if is_final:
                        # denominator rows copied first (tiny), so the rec
                        # DMA + reciprocal run concurrently with the bulk
                        # accumulator copies
                        nc.vector.tensor_copy(
                            osb[D : D + 1, he],
                            po_e[D : D + 1, h * hw_ : (h + 1) * hw_],
                        )
                        nc.vector.tensor_copy(
                            osb[D : D + 1, ho],
                            po_o[D : D + 1, h * hw_ : (h + 1) * hw_],
                        )
                        nc.sync.dma_start(rec[0:1, hfull], osb[D : D + 1, hfull])
                        nc.vector.tensor_copy(
                            osb[0:D, he], po_e[0:D, h * hw_ : (h + 1) * hw_]
                        )
                        nc.vector.tensor_copy(
                            osb[0:D, ho], po_o[0:D, h * hw_ : (h + 1) * hw_]
                        )
                    else:
                        nc.vector.tensor_copy(
                            osb[0 : D + 1, he],
                            po_e[0 : D + 1, h * hw_ : (h + 1) * hw_],
                        )
                        nc.vector.tensor_copy(
                            osb[0 : D + 1, ho],
                            po_o[0 : D + 1, h * hw_ : (h + 1) * hw_],
                        )
                        nc.sync.dma_start(rec[0:1, hfull], osb[D : D + 1, hfull])
                    nc.vector.reciprocal_approx_fast(
                        rcs[0:1, hfull], rec[0:1, hfull]
                    )
                    nc.gpsimd.partition_broadcast(rb[0:D, hfull], rcs[0:1, hfull])
                    nc.vector.tensor_tensor(
                        yT_sb[0:D, pr, hsl], osb[0:D, he], rb[0:D, he], MULT
                    )
                    nc.vector.tensor_tensor(
                        tmp[:, h * hw_ : (h + 1) * hw_],
                        osb[0:D, ho],
                        rb[0:D, ho],
                        MULT,
                    )
                    if not is_final:
                        # odd head's y^T lives on partitions 64-127:
                        # cross-partition move must go through DMA
                        nc.sync.dma_start(
                            yT_sb[D:P, pr, hsl], tmp[:, h * hw_ : (h + 1) * hw_]
                        )
                    else:
                        for tt in range(4 * it + 2 * h, 4 * it + 2 * h + 2):
                            for oc in range(NOC):
                                emit_cproj_split(tt, oc, tmp=tmp)
                if pr == 0 and it + 1 < NIT:
                    # rest of pair 0's own q/k/V tiles, just in time
                    emit_qkv_group(0, it + 1)
                    emit_qkv_group(0, NIT + it + 1)
                    for tt in range(4 * (it + 1), 4 * (it + 2)):
                        emit_v_tile(tt)
                if pr + 1 < JQ:
                    emit_qkv_group(pr + 1, 2 * step)
                    emit_qkv_group(pr + 1, 2 * step + 1)
                # this it-chunk's y^T rows are final: queue their c_proj
                # tiles AFTER this chunk's drains, so every drained tile is
                # one chunk old and its y^T dependency is already satisfied
                # (the final chunk's tiles were already emitted above)
                if not is_final:
                    cproj_queue.extend(
                        (pr, tt * NOC + oc)
                        for tt in range(4 * it, 4 * (it + 1))
                        for oc in range(NOC)
                    )

        while cproj_queue:
            pr_, idx = cproj_queue.pop(0)
            emit_cproj_tile(pr_, idx, tail=True)


def build_nc():
    nc = bacc.Bacc("TRN2", target_bir_lowering=False, debug=False)
    io = {
        "xT": nc.dram_tensor("xT", [C, T], BF16, kind="ExternalInput").ap(),
        "wqkv": nc.dram_tensor("wqkv", [C, 3 * CL], BF16, kind="ExternalInput").ap(),
        "bqk": nc.dram_tensor("bqk", [P, 2 * JQ], FP32, kind="ExternalInput").ap(),
        "bv": nc.dram_tensor("bv", [1, CL], FP32, kind="ExternalInput").ap(),
        "wp": nc.dram_tensor("wp", [CL, C], BF16, kind="ExternalInput").ap(),
        "mtri": nc.dram_tensor("mtri", [P, P], BF16, kind="ExternalInput").ap(),
        "idn": nc.dram_tensor("idn", [P, P], BF16, kind="ExternalInput").ap(),
        # one partial [T, C] per head pair; the host sums them (cheaper
        # than DMA-accumulate, which runs far below line rate).  bf16
        # partials halve the output DMA; the host accumulates in fp32.
        "out": nc.dram_tensor("out", [JQ, T, C], BF16, kind="ExternalOutput").ap(),
    }
    with tile.TileContext(nc) as tc:
        _emit(tc, io)
    # The act-table-load pass assigns each activation the FIRST table set
    # containing its function, so Exp->'exp_and_others' and
    # Ln->'natural_log' alternate (a 1.3us ACT_TABLE_LOAD per switch, ~50
    # switches).  Restrict the choice to 'natural_log_exp_and_others'
    # (which holds every function this kernel uses) so exactly one table
    # load is emitted.  Set ids stay aligned with act_info.json because
    # the dict keeps all entries in order.
    orig_tables = bacc.get_activation_tables

    def _combined_only(arch):
        t = orig_tables(arch)
        return {
            name: (funcs if name == "natural_log_exp_and_others" else set())
            for name, funcs in t.items()
        }

    bacc.get_activation_tables = _combined_only
    try:
        nc.compile()
    finally:
        bacc.get_activation_tables = orig_tables
    return nc


def make_in_maps(x, W_attn, b_attn, W_proj):
    """Per-core input dicts: core = 2*batch + head_group."""
    bf = ml_dtypes.bfloat16
    scale = np.float32(1.0 / math.sqrt(D))
    mtri = np.where(
        np.arange(P)[None, :] < np.arange(P)[:, None],
        np.float32(NEG),
        np.float32(0.0),
    ).astype(bf)
    idn = np.eye(P, dtype=bf)
    in_maps = []
    for core in range(NCORES):
        b, g = divmod(core, 2)
        hs = slice(g * CL, (g + 1) * CL)
        wq = (W_attn[:, 0:C][:, hs] * scale).astype(bf)
        wk = W_attn[:, C : 2 * C][:, hs].astype(bf)
        wv = W_attn[:, 2 * C : 3 * C][:, hs].astype(bf)
        bq = (b_attn[0:C][hs] * scale).astype(np.float32)
        bk = b_attn[C : 2 * C][hs].astype(np.float32)
        bv = b_attn[2 * C : 3 * C][hs].astype(np.float32)
        in_maps.append(
            {
                "xT": np.ascontiguousarray(x[b].T).astype(bf),
                "wqkv": np.ascontiguousarray(np.concatenate([wq, wk, wv], axis=1)),
                "bqk": np.ascontiguousarray(
                    np.concatenate([bq, bk]).reshape(2 * JQ, P).T
                ),
                "bv": bv.reshape(1, CL),
                "wp": np.ascontiguousarray(W_proj[hs, :]).astype(bf),
                "mtri": mtri,
                "idn": idn,
            }
        )
    return in_maps


def combine_outputs(results, b_proj):
    out = np.empty((B, T, C), np.float32)
    for b in range(B):
        acc = results[2 * b]["out"].astype(np.float32).sum(axis=0)
        acc += results[2 * b + 1]["out"].astype(np.float32).sum(axis=0)
        acc += b_proj.astype(np.float32)[None, :]
        out[b] = acc
    return out


def _mask_is_causal(mask):
    if mask.shape != (B, T, T):
        return False
    tril = np.tril(np.ones((T, T), np.float32))
    return all(np.array_equal(np.asarray(mask[b]), tril) for b in range(B))


def _numpy_fallback(x, mask, W_attn, b_attn, W_proj, b_proj):
    # generic-mask fallback (never hit for the causal reference inputs)
    out = np.empty((B, T, C), np.float32)
    for b in range(B):
        qkv = x[b] @ W_attn + b_attn
        q, k, v = np.split(qkv, 3, axis=-1)
        q = q.reshape(T, H, D)
        k = k.reshape(T, H, D)
        v = v.reshape(T, H, D)
        y = np.empty((T, H, D), np.float32)
        for h in range(H):
            s = (q[:, h] @ k[:, h].T) / math.sqrt(D)
            s = s + NEG * (1.0 - mask[b])
            s = s - s.max(-1, keepdims=True)
            p = np.exp(s)
            p /= p.sum(-1, keepdims=True)
            y[:, h] = p @ v[:, h]
        out[b] = y.reshape(T, C) @ W_proj + b_proj
    return out


_NC = None


def kernel(x, mask, W_attn, b_attn, W_proj, b_proj):
    global _NC
    x = np.asarray(x, dtype=np.float32)
    mask = np.asarray(mask)
    W_attn = np.asarray(W_attn, dtype=np.float32)
    b_attn = np.asarray(b_attn, dtype=np.float32)
    W_proj = np.asarray(W_proj, dtype=np.float32)
    b_proj = np.asarray(b_proj, dtype=np.float32)

    if not _mask_is_causal(mask):
        return _numpy_fallback(x, mask, W_attn, b_attn, W_proj, b_proj)

    if _NC is None:
        _NC = build_nc()
    in_maps = make_in_maps(x, W_attn, b_attn, W_proj)
    res = run_bass_kernel_spmd(_NC, in_maps, core_ids=list(range(NCORES)))
    return combine_outputs(res.results, b_proj)

